# revision 4
# baseline (speedup 1.0000x reference)
"""Trainium2 Bass kernel for nn_DeformableSVDModulatedConv2d.

Strategy (data-parallel over batch, 8 cores x 2 samples):
  per sample b on each core:
    delta[m,o] = sum_r u[m,r] * (ev_b[r] * vh[r,o])   (m=(ky,kx,cin), 36 m-tiles)
    norm2 = sum delta^2 ; alpha = shift_b / max(sqrt(norm2),1e-12)
    wgt[m,o] = W[m,o] + alpha*delta[m,o]              (W host-permuted to [m,o])
    q[o] = sum_m s2_b[m] * wgt[m,o]^2 ; demod = SCALE/sqrt(SCALE^2 q + 1e-8)
    out[o,y,x] = demod[o] * sum_{ky,kx,cin} wgt.T conv (s_b * x_b)   (36 shifted
                 matmuls per (o-tile, row-half) accumulated in PSUM)
Compute dtype bf16 on the PE (fp32 PSUM accumulation), fp32 everywhere scalar.
"""
import os
import sys
import types

if '/opt/trn_rl_repo' not in sys.path:
    sys.path.insert(0, '/opt/trn_rl_repo')

import numpy as np
import ml_dtypes

import concourse.bass as bass
import concourse.mybir as mybir
import concourse.tile as tile
from concourse.bass_utils import run_bass_kernel_spmd

F32 = mybir.dt.float32
BF16 = mybir.dt.bfloat16
BF = ml_dtypes.bfloat16

B, CIN, COUT, K, H, W = 16, 512, 512, 3, 32, 32
SDIM, NDIR, R = 512, 64, 512
SCALE = 1.0 / np.sqrt(CIN * K * K)
NCORES = 8
LB = B // NCORES          # samples per core
M = K * K * CIN           # 4608
NJ = M // 128             # 36 m-tiles
NRC = R // 128            # 4 r-chunks
NC_CH = CIN // 128        # 4 cin chunks
NOC = COUT // 128         # 4 cout chunks
WP = W + 2                # 34 padded cols

Alu = mybir.AluOpType
Act = mybir.ActivationFunctionType


def _install_ntff_hook():
    """Optional: register the axon NTFF profiling hook (image's antenv lacks it)."""
    try:
        import antenv
        if 'antenv.axon_hooks' in sys.modules:
            return
        mod = types.ModuleType('antenv.axon_hooks')
        _h = [None]
        mod.set_axon_ntff_profile_hook = lambda h: _h.__setitem__(0, h)
        mod.get_axon_ntff_profile_hook = lambda: _h[0]
        sys.modules['antenv.axon_hooks'] = mod
        antenv.axon_hooks = mod
        from trn_agent_boot.trn_boot import _ntff_profile_via_ctypes
        mod.set_axon_ntff_profile_hook(
            _ntff_profile_via_ctypes('/opt/axon/libaxon_pjrt.so'))
    except Exception:
        pass


def _split_waits(nc, maxw=1):
    """walrus CoreV3 rejects >~4 sem waits on one instruction (Tile tail Drain).
    Move excess waits onto preceding same-engine NoOps."""
    cnt = 0
    for f in nc.m.functions:
        for bb in f.blocks:
            new_insts = []
            for inst in bb.instructions:
                si = inst.sync_info
                if si is not None and si.on_wait and len(si.on_wait) > maxw:
                    waits = list(si.on_wait)
                    for wt in waits[:-maxw]:
                        cnt += 1
                        new_insts.append(mybir.InstNoOp(
                            name=f"waitsplit-{cnt}", ins=[], outs=[],
                            engine=inst.engine,
                            sync_info=mybir.SyncInfo(on_wait=[wt], on_update=[])))
                    si.on_wait = waits[-maxw:]
                new_insts.append(inst)
            bb.instructions[:] = new_insts
    return cnt


def _row_range(h, ky):
    """Output rows covered by tap row ky within half h -> (y0, nrows)."""
    y0 = max(16 * h, 1 - ky + 0)
    y1 = min(16 * h + 15, 31 + 1 - ky)
    return y0, y1 - y0 + 1


def build_program():
    nc = bass.Bass()
    ut = nc.declare_dram_parameter("ut", [R, M], BF16, isOutput=False)
    wm = nc.declare_dram_parameter("wm", [M, COUT], BF16, isOutput=False)
    vh = nc.declare_dram_parameter("vh", [R, COUT], F32, isOutput=False)
    mwt = nc.declare_dram_parameter("mwt", [SDIM, CIN], F32, isOutput=False)
    mb = nc.declare_dram_parameter("mb", [CIN], F32, isOutput=False)
    stl = nc.declare_dram_parameter("stl", [SDIM, LB], F32, isOutput=False)
    ev = nc.declare_dram_parameter("ev", [R, LB], F32, isOutput=False)
    sh = nc.declare_dram_parameter("sh", [LB], F32, isOutput=False)
    xin = nc.declare_dram_parameter("x", [LB, CIN, H, W], F32, isOutput=False)
    out = nc.declare_dram_parameter("out", [LB, COUT, H, W], F32, isOutput=True)

    ut_r = ut.rearrange("(rc p) (j m) -> p rc j m", p=128, m=128)
    wm_r = wm.rearrange("(j p) o -> p j o", p=128)
    vh_r = vh.rearrange("(rc p) o -> p rc o", p=128)
    ev_r = ev.rearrange("(rc p) b -> p rc b", p=128)
    stl_r = stl.rearrange("(dc p) b -> p dc b", p=128)
    mb_r = mb.rearrange("(c p) -> p c", p=128)
    sh_r = sh.rearrange("(a b) -> a b", a=1)

    with tile.TileContext(nc) as tc:
        from contextlib import ExitStack
        with ExitStack() as ctx:
            p_const = ctx.enter_context(tc.tile_pool(name="const", bufs=1))
            p_w = ctx.enter_context(tc.tile_pool(name="pw", bufs=1))
            p_in = ctx.enter_context(tc.tile_pool(name="pin", bufs=1))
            p_mwt = ctx.enter_context(tc.tile_pool(name="pmwt", bufs=4))
            p_u = ctx.enter_context(tc.tile_pool(name="pu", bufs=3))
            p_xpad = ctx.enter_context(tc.tile_pool(name="pxpad", bufs=2))
            p_xs = ctx.enter_context(tc.tile_pool(name="pxs", bufs=8))
            p_evh = ctx.enter_context(tc.tile_pool(name="pevh", bufs=8))
            p_d = ctx.enter_context(tc.tile_pool(name="pd", bufs=NJ))
            p_wgt = ctx.enter_context(tc.tile_pool(name="pwgt", bufs=NJ))
            p_sq = ctx.enter_context(tc.tile_pool(name="psq", bufs=3))
            p_ob = ctx.enter_context(tc.tile_pool(name="pob", bufs=3))
            p_sm = ctx.enter_context(tc.tile_pool(name="psm", bufs=2))
            ps_conv = ctx.enter_context(
                tc.tile_pool(name="psconv", bufs=3, space="PSUM"))
            ps_d = ctx.enter_context(
                tc.tile_pool(name="psd", bufs=2, space="PSUM"))
            ps_q = ctx.enter_context(
                tc.tile_pool(name="psq2", bufs=1, space="PSUM"))
            ps_sm = ctx.enter_context(
                tc.tile_pool(name="pssm", bufs=2, space="PSUM"))

            # constants
            ones128 = p_const.tile([128, 1], F32, name="ones128")
            nc.vector.memset(ones128[:], 1.0)
            ones1x = p_const.tile([1, 128], F32, name="ones1x")
            nc.vector.memset(ones1x[:], 1.0)
            id1 = p_const.tile([1, 1], F32, name="id1")
            nc.vector.memset(id1[:], 1.0)
            eps8 = p_const.tile([1, 1], F32, name="eps8")
            nc.vector.memset(eps8[:], 1e-8)

            # bulk loads
            w_all = p_w.tile([128, NJ, 512], BF16, name="w_all")
            nc.sync.dma_start(out=w_all[:], in_=wm_r)
            vh_sb = p_in.tile([128, NRC, 512], F32, name="vh_sb")
            nc.sync.dma_start(out=vh_sb[:], in_=vh_r)
            ev_sb = p_in.tile([128, NRC, LB], F32, name="ev_sb")
            nc.sync.dma_start(out=ev_sb[:], in_=ev_r)
            stl_sb = p_in.tile([128, NRC, LB], F32, name="stl_sb")
            nc.sync.dma_start(out=stl_sb[:], in_=stl_r)
            mb_sb = p_in.tile([128, NC_CH], F32, name="mb_sb")
            nc.sync.dma_start(out=mb_sb[:], in_=mb_r)
            sh_sb = p_in.tile([1, LB], F32, name="sh_sb")
            nc.sync.dma_start(out=sh_sb[:], in_=sh_r)

            # style modulation s = style @ mw.T + mb  -> [128(i), LB] per chunk
            mwt_t = []
            for dc in range(NRC):
                t = p_mwt.tile([128, 512], F32, name=f"mwt{dc}", tag="mwt")
                nc.sync.dma_start(out=t[:], in_=mwt[dc * 128:(dc + 1) * 128, :])
                mwt_t.append(t)
            s_sb, s2_sb = [], []
            for ic in range(NC_CH):
                ps = ps_sm.tile([128, LB], F32, name=f"ps_s{ic}", tag="pssm")
                for dc in range(NRC):
                    nc.tensor.matmul(ps[:], mwt_t[dc][:, ic * 128:(ic + 1) * 128],
                                     stl_sb[:, dc, :],
                                     start=(dc == 0), stop=(dc == NRC - 1))
                s_t = p_in.tile([128, LB], F32, name=f"s{ic}")
                nc.vector.tensor_scalar_add(s_t[:], ps[:], mb_sb[:, ic:ic + 1])
                s2_t = p_in.tile([128, LB], BF16, name=f"s2_{ic}")
                nc.vector.tensor_mul(s2_t[:], s_t[:], s_t[:])
                s_sb.append(s_t)
                s2_sb.append(s2_t)

            # evh[b][rc] = ev_b * vh   (bf16)
            evh = [[None] * NRC for _ in range(LB)]
            for b in range(LB):
                for rc in range(NRC):
                    t = p_evh.tile([128, 512], BF16, name=f"evh{b}_{rc}", tag="evh")
                    nc.vector.tensor_scalar_mul(t[:], vh_sb[:, rc, :],
                                                ev_sb[:, rc, b:b + 1])
                    evh[b][rc] = t

            # x load + pad cols + modulate by s -> bf16
            xs = [[None] * NC_CH for _ in range(LB)]
            for b in range(LB):
                for c in range(NC_CH):
                    xp = p_xpad.tile([128, H, WP], F32, name=f"xp{b}{c}", tag="xpad")
                    nc.gpsimd.memset(xp[:], 0.0)
                    nc.sync.dma_start(out=xp[:, :, 1:33],
                                      in_=xin[b, c * 128:(c + 1) * 128, :, :])
                    t = p_xs.tile([128, H, WP], BF16, name=f"xs{b}{c}", tag="xs")
                    nc.vector.tensor_scalar_mul(t[:], xp[:], s_sb[c][:, b:b + 1])
                    xs[b][c] = t

            for b in range(LB):
                # ---- delta matmuls + norm partials ----
                nacc = p_sm.tile([128, NJ], F32, name=f"nacc{b}", tag="nacc")
                deltas = []
                for j in range(NJ):
                    uj = p_u.tile([128, NRC, 128], BF16, name=f"u{b}_{j}", tag="uj")
                    nc.sync.dma_start(out=uj[:], in_=ut_r[:, :, j, :])
                    pd = ps_d.tile([128, 512], F32, name=f"pd{b}_{j}", tag="pd")
                    for rc in range(NRC):
                        nc.tensor.matmul(pd[:], uj[:, rc, :], evh[b][rc][:],
                                         start=(rc == 0), stop=(rc == NRC - 1))
                    dj = p_d.tile([128, 512], BF16, name=f"d{b}_{j}", tag="delta")
                    nc.vector.tensor_copy(dj[:], pd[:])
                    scr = p_sq.tile([128, 512], BF16, name=f"nsq{b}_{j}", tag="sq")
                    nc.scalar.activation(scr[:], dj[:], Act.Square,
                                         accum_out=nacc[:, j:j + 1])
                    deltas.append(dj)

                # ---- alpha = shift / max(norm, 1e-12), broadcast to [128,1] ----
                nred = p_sm.tile([128, 1], F32, name=f"nred{b}", tag="nred")
                nc.vector.reduce_sum(nred[:], nacc[:], axis=mybir.AxisListType.X)
                pn = ps_sm.tile([1, 1], F32, name=f"pn{b}", tag="pssm")
                nc.tensor.matmul(pn[:], nred[:], ones128[:], start=True, stop=True)
                norm_s = p_sm.tile([1, 1], F32, name=f"norm{b}", tag="n1")
                nc.scalar.sqrt(norm_s[:], pn[:])
                nc.vector.tensor_scalar_max(norm_s[:], norm_s[:], 1e-12)
                rnorm = p_sm.tile([1, 1], F32, name=f"rn{b}", tag="n2")
                nc.vector.reciprocal(rnorm[:], norm_s[:])
                al1 = p_sm.tile([1, 1], F32, name=f"al{b}", tag="n3")
                nc.vector.tensor_mul(al1[:], rnorm[:], sh_sb[:, b:b + 1])
                pa = ps_sm.tile([128, 1], F32, name=f"pa{b}", tag="pssm")
                nc.tensor.matmul(pa[:], ones1x[:], al1[:], start=True, stop=True)
                al_bc = p_sm.tile([128, 1], F32, name=f"albc{b}", tag="n4")
                nc.vector.tensor_copy(al_bc[:], pa[:])

                # ---- wgt = W + alpha*delta ; q[o] = sum s2*wgt^2 ----
                pq = ps_q.tile([1, 512], F32, name=f"pq{b}", tag="pq")
                wgts = []
                for j in range(NJ):
                    wj = p_wgt.tile([128, 512], BF16, name=f"w{b}_{j}", tag="wgt")
                    nc.vector.scalar_tensor_tensor(
                        wj[:], in0=deltas[j][:], scalar=al_bc[:],
                        in1=w_all[:, j, :], op0=Alu.mult, op1=Alu.add)
                    sq = p_sq.tile([128, 512], BF16, name=f"sq{b}_{j}", tag="sq")
                    nc.vector.tensor_mul(sq[:], wj[:], wj[:])
                    nc.tensor.matmul(pq[:], s2_sb[j % NC_CH][:, b:b + 1], sq[:],
                                     start=(j == 0), stop=(j == NJ - 1))
                    wgts.append(wj)

                # ---- demod = SCALE / sqrt(SCALE^2 q + 1e-8), to [128, NOC] ----
                dmf = p_sm.tile([1, 512], F32, name=f"dmf{b}", tag="dmf")
                nc.scalar.activation(dmf[:], pq[:], Act.Sqrt,
                                     bias=eps8[:], scale=float(SCALE * SCALE))
                dm2 = p_sm.tile([1, 512], F32, name=f"dm2{b}", tag="dm2")
                nc.vector.reciprocal(dm2[:], dmf[:])
                dm3 = p_sm.tile([1, 512], F32, name=f"dm3{b}", tag="dm3")
                nc.vector.tensor_scalar_mul(dm3[:], dm2[:], float(SCALE))
                dmt = p_sm.tile([128, NOC], F32, name=f"dmt{b}", tag="dmt")
                for oc in range(NOC):
                    nc.sync.dma_start(
                        out=dmt[:, oc:oc + 1],
                        in_=dm3[:, oc * 128:(oc + 1) * 128])

                # ---- conv: 36 shifted matmuls per (oc, half), PSUM accumulate ----
                for oc in range(NOC):
                    for hf in range(2):
                        pc = ps_conv.tile([128, 16, 32], F32,
                                          name=f"pc{b}{oc}{hf}", tag="pc")
                        first = True
                        for t in range(K * K):
                            ky, kx = t // K, t % K
                            y0, nr = _row_range(hf, ky)
                            ry0 = y0 + ky - 1
                            yl = y0 - 16 * hf
                            for c in range(NC_CH):
                                j = t * NC_CH + c
                                nc.tensor.matmul(
                                    pc[:, yl:yl + nr, :],
                                    wgts[j][:, oc * 128:(oc + 1) * 128],
                                    xs[b][c][:, ry0:ry0 + nr, kx:kx + 32],
                                    start=first,
                                    stop=(t == K * K - 1 and c == NC_CH - 1))
                                first = False
                        ob = p_ob.tile([128, 16, 32], F32,
                                       name=f"ob{b}{oc}{hf}", tag="ob")
                        nc.vector.tensor_scalar_mul(ob[:], pc[:],
                                                    dmt[:, oc:oc + 1])
                        nc.sync.dma_start(
                            out=out[b, oc * 128:(oc + 1) * 128,
                                    hf * 16:hf * 16 + 16, :],
                            in_=ob[:])
    _split_waits(nc)
    return nc


_CACHED = {}


def _get_program():
    if 'nc' not in _CACHED:
        _CACHED['nc'] = build_program()
    return _CACHED['nc']


def kernel(x, style, modulation_w, modulation_b, weight, u, vh,
           dir_delta, batch_shifts, batch_directions):
    x = np.asarray(x, dtype=np.float32)
    style = np.asarray(style, dtype=np.float32)
    modulation_w = np.asarray(modulation_w, dtype=np.float32)
    modulation_b = np.asarray(modulation_b, dtype=np.float32)
    weight = np.asarray(weight, dtype=np.float32)
    vh = np.asarray(vh, dtype=np.float32)
    u = np.asarray(u, dtype=np.float32)
    dir_delta = np.asarray(dir_delta, dtype=np.float32)
    batch_shifts = np.asarray(batch_shifts, dtype=np.float32)
    bd = np.asarray(batch_directions).astype(np.int64)

    ut_h = np.ascontiguousarray(u.T).astype(BF)                       # [R, M]
    wm_h = np.ascontiguousarray(
        weight.transpose(2, 3, 1, 0).reshape(M, COUT)).astype(BF)     # [m, o]
    mwt_h = np.ascontiguousarray(modulation_w.T)                      # [d, i]
    stl_h = np.ascontiguousarray(style.T)                             # [d, B]
    ev_h = np.ascontiguousarray(dir_delta[bd].T)                      # [R, B]

    in_maps = []
    for cid in range(NCORES):
        sl = slice(cid * LB, (cid + 1) * LB)
        in_maps.append({
            "ut": ut_h, "wm": wm_h, "vh": vh, "mwt": mwt_h,
            "mb": modulation_b,
            "stl": np.ascontiguousarray(stl_h[:, sl]),
            "ev": np.ascontiguousarray(ev_h[:, sl]),
            "sh": np.ascontiguousarray(batch_shifts[sl]),
            "x": np.ascontiguousarray(x[sl]),
        })

    nc = _get_program()
    trace = os.environ.get("BASS_KERNEL_TRACE", "") == "1"
    if trace:
        _install_ntff_hook()
    res = run_bass_kernel_spmd(nc, in_maps, list(range(NCORES)), trace=trace)
    if trace:
        kernel.last_exec_time_ns = res.exec_time_ns
    outs = [res.results[i]["out"] for i in range(NCORES)]
    return np.concatenate(outs, axis=0)


kernel.last_exec_time_ns = None


# revision 9
# speedup vs baseline: 1.1584x; 1.1584x over previous
"""Trainium2 Bass kernel for nn_DeformableSVDModulatedConv2d.

Strategy (data-parallel over batch, 8 cores x 2 samples):
  per sample b on each core:
    delta[m,o] = sum_r u[m,r] * (ev_b[r] * vh[r,o])   (m=(ky,kx,cin), 36 m-tiles)
    norm2 = sum delta^2 ; alpha = shift_b / max(sqrt(norm2),1e-12)
    wgt[m,o] = W[m,o] + alpha*delta[m,o]              (W host-permuted to [m,o])
    q[o] = sum_m s2_b[m] * wgt[m,o]^2 ; demod = SCALE/sqrt(SCALE^2 q + 1e-8)
    out[o,y,x] = demod[o] * sum_{ky,kx,cin} wgt.T conv (s_b * x_b)   (36 shifted
                 matmuls per (o-tile, row-half) accumulated in PSUM)
Compute dtype bf16 on the PE (fp32 PSUM accumulation), fp32 everywhere scalar.
"""
import os
import sys
import types

if '/opt/trn_rl_repo' not in sys.path:
    sys.path.insert(0, '/opt/trn_rl_repo')

import numpy as np
import ml_dtypes

import concourse.bass as bass
import concourse.mybir as mybir
import concourse.tile as tile
from concourse.bass_utils import run_bass_kernel_spmd

F32 = mybir.dt.float32
BF16 = mybir.dt.bfloat16
BF = ml_dtypes.bfloat16

B, CIN, COUT, K, H, W = 16, 512, 512, 3, 32, 32
SDIM, NDIR, R = 512, 64, 512
SCALE = 1.0 / np.sqrt(CIN * K * K)
NCORES = 8
LB = B // NCORES          # samples per core
M = K * K * CIN           # 4608
NJ = M // 128             # 36 m-tiles
NRC = R // 128            # 4 r-chunks
NC_CH = CIN // 128        # 4 cin chunks
NOC = COUT // 128         # 4 cout chunks
WP = W + 2                # 34 padded cols

Alu = mybir.AluOpType
Act = mybir.ActivationFunctionType


def _install_ntff_hook():
    """Optional: register the axon NTFF profiling hook (image's antenv lacks it)."""
    try:
        import antenv
        if 'antenv.axon_hooks' in sys.modules:
            return
        mod = types.ModuleType('antenv.axon_hooks')
        _h = [None]
        mod.set_axon_ntff_profile_hook = lambda h: _h.__setitem__(0, h)
        mod.get_axon_ntff_profile_hook = lambda: _h[0]
        sys.modules['antenv.axon_hooks'] = mod
        antenv.axon_hooks = mod
        from trn_agent_boot.trn_boot import _ntff_profile_via_ctypes
        mod.set_axon_ntff_profile_hook(
            _ntff_profile_via_ctypes('/opt/axon/libaxon_pjrt.so'))
    except Exception:
        pass


def _split_waits(nc, maxw=1):
    """walrus CoreV3 rejects >~4 sem waits on one instruction (Tile tail Drain).
    Move excess waits onto preceding same-engine NoOps."""
    cnt = 0
    for f in nc.m.functions:
        for bb in f.blocks:
            new_insts = []
            for inst in bb.instructions:
                si = inst.sync_info
                if si is not None and si.on_wait and len(si.on_wait) > maxw:
                    waits = list(si.on_wait)
                    for wt in waits[:-maxw]:
                        cnt += 1
                        new_insts.append(mybir.InstNoOp(
                            name=f"waitsplit-{cnt}", ins=[], outs=[],
                            engine=inst.engine,
                            sync_info=mybir.SyncInfo(on_wait=[wt], on_update=[])))
                    si.on_wait = waits[-maxw:]
                new_insts.append(inst)
            bb.instructions[:] = new_insts
    return cnt


def _row_range(h, ky):
    """Output rows covered by tap row ky within half h -> (y0, nrows)."""
    y0 = max(16 * h, 1 - ky + 0)
    y1 = min(16 * h + 15, 31 + 1 - ky)
    return y0, y1 - y0 + 1


def build_program():
    nc = bass.Bass()
    ut = nc.declare_dram_parameter("ut", [R, M], BF16, isOutput=False)
    wm = nc.declare_dram_parameter("wm", [M, COUT], BF16, isOutput=False)
    vh = nc.declare_dram_parameter("vh", [R, COUT], F32, isOutput=False)
    mwt = nc.declare_dram_parameter("mwt", [SDIM, CIN], F32, isOutput=False)
    mb = nc.declare_dram_parameter("mb", [CIN], F32, isOutput=False)
    stl = nc.declare_dram_parameter("stl", [SDIM, LB], F32, isOutput=False)
    ev = nc.declare_dram_parameter("ev", [R, LB], F32, isOutput=False)
    sh = nc.declare_dram_parameter("sh", [LB], F32, isOutput=False)
    xin = nc.declare_dram_parameter("x", [LB, CIN, H, W], F32, isOutput=False)
    out = nc.declare_dram_parameter("out", [LB, COUT, H, W], F32, isOutput=True)

    ut_r = ut.rearrange("(rc p) (j m) -> p rc j m", p=128, m=128)
    wm_r = wm.rearrange("(j p) o -> p j o", p=128)
    vh_r = vh.rearrange("(rc p) o -> p rc o", p=128)
    ev_r = ev.rearrange("(rc p) b -> p rc b", p=128)
    stl_r = stl.rearrange("(dc p) b -> p dc b", p=128)
    mb_r = mb.rearrange("(c p) -> p c", p=128)
    sh_r = sh.rearrange("(a b) -> a b", a=1)

    with tile.TileContext(nc) as tc:
        from contextlib import ExitStack
        with ExitStack() as ctx:
            p_const = ctx.enter_context(tc.tile_pool(name="const", bufs=1))
            p_w = ctx.enter_context(tc.tile_pool(name="pw", bufs=1))
            p_in = ctx.enter_context(tc.tile_pool(name="pin", bufs=1))
            p_mwt = ctx.enter_context(tc.tile_pool(name="pmwt", bufs=4))
            p_u = ctx.enter_context(tc.tile_pool(name="pu", bufs=8))
            p_xpad = ctx.enter_context(tc.tile_pool(name="pxpad", bufs=2))
            p_xs = ctx.enter_context(tc.tile_pool(name="pxs", bufs=8))
            p_evh = ctx.enter_context(tc.tile_pool(name="pevh", bufs=8))
            p_d = ctx.enter_context(tc.tile_pool(name="pd", bufs=NJ + 6))
            p_wgt = ctx.enter_context(tc.tile_pool(name="pwgt", bufs=NJ))
            p_sq = ctx.enter_context(tc.tile_pool(name="psq", bufs=3))
            p_ob = ctx.enter_context(tc.tile_pool(name="pob", bufs=3))
            p_sm = ctx.enter_context(tc.tile_pool(name="psm", bufs=2))
            ps_conv = ctx.enter_context(
                tc.tile_pool(name="psconv", bufs=3, space="PSUM"))
            ps_d = ctx.enter_context(
                tc.tile_pool(name="psd", bufs=3, space="PSUM"))
            ps_sm = ctx.enter_context(
                tc.tile_pool(name="pssm", bufs=2, space="PSUM"))

            # constants
            ones128 = p_const.tile([128, 1], F32, name="ones128")
            nc.vector.memset(ones128[:], 1.0)
            ones1x = p_const.tile([1, 128], F32, name="ones1x")
            nc.vector.memset(ones1x[:], 1.0)
            id1 = p_const.tile([1, 1], F32, name="id1")
            nc.vector.memset(id1[:], 1.0)
            eps8 = p_const.tile([1, 1], F32, name="eps8")
            nc.vector.memset(eps8[:], 1e-8)

            # small loads first (w_all + x are emitted after sample-0's delta
            # loop so the u-tile stream gets DMA priority at kernel start)
            vh_sb = p_in.tile([128, NRC, 512], F32, name="vh_sb")
            nc.sync.dma_start(out=vh_sb[:], in_=vh_r)
            ev_sb = p_in.tile([128, NRC, LB], F32, name="ev_sb")
            nc.sync.dma_start(out=ev_sb[:], in_=ev_r)
            stl_sb = p_in.tile([128, NRC, LB], F32, name="stl_sb")
            nc.sync.dma_start(out=stl_sb[:], in_=stl_r)
            mb_sb = p_in.tile([128, NC_CH], F32, name="mb_sb")
            nc.sync.dma_start(out=mb_sb[:], in_=mb_r)
            sh_sb = p_in.tile([1, LB], F32, name="sh_sb")
            nc.sync.dma_start(out=sh_sb[:], in_=sh_r)

            # style modulation s = style @ mw.T + mb  -> [128(i), LB] per chunk
            mwt_t = []
            for dc in range(NRC):
                t = p_mwt.tile([128, 512], F32, name=f"mwt{dc}", tag="mwt")
                nc.sync.dma_start(out=t[:], in_=mwt[dc * 128:(dc + 1) * 128, :])
                mwt_t.append(t)
            s_sb = []
            for ic in range(NC_CH):
                ps = ps_sm.tile([128, LB], F32, name=f"ps_s{ic}", tag="pssm")
                for dc in range(NRC):
                    nc.tensor.matmul(ps[:], mwt_t[dc][:, ic * 128:(ic + 1) * 128],
                                     stl_sb[:, dc, :],
                                     start=(dc == 0), stop=(dc == NRC - 1))
                s_t = p_in.tile([128, LB], F32, name=f"s{ic}")
                nc.vector.tensor_scalar_add(s_t[:], ps[:], mb_sb[:, ic:ic + 1])
                s_sb.append(s_t)

            # evh[b][rc] = ev_b * vh   (bf16)
            evh = [[None] * NRC for _ in range(LB)]
            for b in range(LB):
                for rc in range(NRC):
                    t = p_evh.tile([128, 512], BF16, name=f"evh{b}_{rc}", tag="evh")
                    nc.vector.tensor_scalar_mul(t[:], vh_sb[:, rc, :],
                                                ev_sb[:, rc, b:b + 1])
                    evh[b][rc] = t

            xs = [[None] * NC_CH for _ in range(LB)]
            deltas = [[None] * NJ for _ in range(LB)]
            naccs = [None] * LB
            w_all = [None]

            def emit_delta(b):
                # ---- delta matmuls + norm partials ----
                nacc = p_sm.tile([128, NJ], F32, name=f"nacc{b}", tag="nacc")
                naccs[b] = nacc
                for j in range(NJ):
                    uj = p_u.tile([128, NRC, 128], BF16, name=f"u{b}_{j}", tag="uj")
                    nc.sync.dma_start(out=uj[:], in_=ut_r[:, :, j, :])
                    pd = ps_d.tile([128, 512], F32, name=f"pd{b}_{j}", tag="pd")
                    for rc in range(NRC):
                        nc.tensor.matmul(pd[:], uj[:, rc, :], evh[b][rc][:],
                                         start=(rc == 0), stop=(rc == NRC - 1))
                    dj = p_d.tile([128, 512], BF16, name=f"d{b}_{j}", tag="delta")
                    nc.vector.tensor_copy(dj[:], pd[:])
                    scr = p_sq.tile([128, 512], BF16, name=f"nsq{b}_{j}", tag="sq")
                    nc.scalar.activation(scr[:], dj[:], Act.Square,
                                         accum_out=nacc[:, j:j + 1])
                    deltas[b][j] = dj

            def emit_bulk_loads():
                # big shared-weight load + x load/pad/modulate, emitted after
                # sample 0's delta loop so they don't starve the u stream
                wa = p_w.tile([128, NJ, 512], BF16, name="w_all")
                nc.sync.dma_start(out=wa[:], in_=wm_r)
                w_all[0] = wa
                for b in range(LB):
                    for c in range(NC_CH):
                        xp = p_xpad.tile([128, H, WP], F32,
                                         name=f"xp{b}{c}", tag="xpad")
                        nc.gpsimd.memset(xp[:], 0.0)
                        nc.sync.dma_start(out=xp[:, :, 1:33],
                                          in_=xin[b, c * 128:(c + 1) * 128, :, :])
                        t = p_xs.tile([128, H, WP], BF16, name=f"xs{b}{c}", tag="xs")
                        nc.vector.tensor_scalar_mul(t[:], xp[:],
                                                    s_sb[c][:, b:b + 1])
                        xs[b][c] = t

            def emit_rest(b):
                # ---- alpha = shift / max(norm, 1e-12), broadcast to [128,1] ----
                nacc = naccs[b]
                nred = p_sm.tile([128, 1], F32, name=f"nred{b}", tag="nred")
                nc.vector.reduce_sum(nred[:], nacc[:], axis=mybir.AxisListType.X)
                pn = ps_sm.tile([1, 1], F32, name=f"pn{b}", tag="pssm")
                nc.tensor.matmul(pn[:], nred[:], ones128[:], start=True, stop=True)
                norm_s = p_sm.tile([1, 1], F32, name=f"norm{b}", tag="n1")
                nc.scalar.sqrt(norm_s[:], pn[:])
                nc.vector.tensor_scalar_max(norm_s[:], norm_s[:], 1e-12)
                rnorm = p_sm.tile([1, 1], F32, name=f"rn{b}", tag="n2")
                nc.vector.reciprocal(rnorm[:], norm_s[:])
                al1 = p_sm.tile([1, 1], F32, name=f"al{b}", tag="n3")
                nc.vector.tensor_mul(al1[:], rnorm[:], sh_sb[:, b:b + 1])
                pa = ps_sm.tile([128, 1], F32, name=f"pa{b}", tag="pssm")
                nc.tensor.matmul(pa[:], ones1x[:], al1[:], start=True, stop=True)
                al_bc = p_sm.tile([128, 1], F32, name=f"albc{b}", tag="n4")
                nc.vector.tensor_copy(al_bc[:], pa[:])

                # ---- wgt = W + alpha*delta ; qacc[p,o] += (s_p*wgt[p,o])^2 ----
                qacc = p_sm.tile([128, 512], F32, name=f"qacc{b}", tag="qacc")
                wgts = []
                for j in range(NJ):
                    wj = p_wgt.tile([128, 512], BF16, name=f"w{b}_{j}", tag="wgt")
                    nc.vector.scalar_tensor_tensor(
                        wj[:], in0=deltas[b][j][:], scalar=al_bc[:],
                        in1=w_all[0][:, j, :], op0=Alu.mult, op1=Alu.add)
                    sq = p_sq.tile([128, 512], BF16, name=f"sq{b}_{j}", tag="sq")
                    nc.scalar.activation(sq[:], wj[:], Act.Square,
                                         scale=s_sb[j % NC_CH][:, b:b + 1])
                    if j == 0:
                        nc.vector.tensor_copy(qacc[:], sq[:])
                    else:
                        nc.vector.tensor_add(qacc[:], qacc[:], sq[:])
                    wgts.append(wj)
                pq = ps_sm.tile([1, 512], F32, name=f"pq{b}", tag="pssm")
                nc.tensor.matmul(pq[:], ones128[:], qacc[:], start=True, stop=True)

                # ---- demod = SCALE / sqrt(SCALE^2 q + 1e-8), to [128, NOC] ----
                dmf = p_sm.tile([1, 512], F32, name=f"dmf{b}", tag="dmf")
                nc.scalar.activation(dmf[:], pq[:], Act.Sqrt,
                                     bias=eps8[:], scale=float(SCALE * SCALE))
                dm2 = p_sm.tile([1, 512], F32, name=f"dm2{b}", tag="dm2")
                nc.vector.reciprocal(dm2[:], dmf[:])
                dm3 = p_sm.tile([1, 512], F32, name=f"dm3{b}", tag="dm3")
                nc.vector.tensor_scalar_mul(dm3[:], dm2[:], float(SCALE))
                dmt = p_sm.tile([128, NOC], F32, name=f"dmt{b}", tag="dmt")
                for oc in range(NOC):
                    nc.sync.dma_start(
                        out=dmt[:, oc:oc + 1],
                        in_=dm3[:, oc * 128:(oc + 1) * 128])

                # ---- conv: 36 shifted matmuls per (oc, half), PSUM accumulate ----
                for oc in range(NOC):
                    for hf in range(2):
                        pc = ps_conv.tile([128, 16, 32], F32,
                                          name=f"pc{b}{oc}{hf}", tag="pc")
                        first = True
                        for t in range(K * K):
                            ky, kx = t // K, t % K
                            y0, nr = _row_range(hf, ky)
                            ry0 = y0 + ky - 1
                            yl = y0 - 16 * hf
                            for c in range(NC_CH):
                                j = t * NC_CH + c
                                nc.tensor.matmul(
                                    pc[:, yl:yl + nr, :],
                                    wgts[j][:, oc * 128:(oc + 1) * 128],
                                    xs[b][c][:, ry0:ry0 + nr, kx:kx + 32],
                                    start=first,
                                    stop=(t == K * K - 1 and c == NC_CH - 1))
                                first = False
                        ob = p_ob.tile([128, 16, 32], F32,
                                       name=f"ob{b}{oc}{hf}", tag="ob")
                        nc.vector.tensor_scalar_mul(ob[:], pc[:],
                                                    dmt[:, oc:oc + 1])
                        nc.sync.dma_start(
                            out=out[b, oc * 128:(oc + 1) * 128,
                                    hf * 16:hf * 16 + 16, :],
                            in_=ob[:])

            emit_delta(0)
            emit_bulk_loads()
            emit_rest(0)
            emit_delta(1)
            emit_rest(1)
    _split_waits(nc)
    return nc


_CACHED = {}


def _get_program():
    if 'nc' not in _CACHED:
        _CACHED['nc'] = build_program()
    return _CACHED['nc']


def kernel(x, style, modulation_w, modulation_b, weight, u, vh,
           dir_delta, batch_shifts, batch_directions):
    x = np.asarray(x, dtype=np.float32)
    style = np.asarray(style, dtype=np.float32)
    modulation_w = np.asarray(modulation_w, dtype=np.float32)
    modulation_b = np.asarray(modulation_b, dtype=np.float32)
    weight = np.asarray(weight, dtype=np.float32)
    vh = np.asarray(vh, dtype=np.float32)
    u = np.asarray(u, dtype=np.float32)
    dir_delta = np.asarray(dir_delta, dtype=np.float32)
    batch_shifts = np.asarray(batch_shifts, dtype=np.float32)
    bd = np.asarray(batch_directions).astype(np.int64)

    ut_h = np.ascontiguousarray(u.T).astype(BF)                       # [R, M]
    wm_h = np.ascontiguousarray(
        weight.transpose(2, 3, 1, 0).reshape(M, COUT)).astype(BF)     # [m, o]
    mwt_h = np.ascontiguousarray(modulation_w.T)                      # [d, i]
    stl_h = np.ascontiguousarray(style.T)                             # [d, B]
    ev_h = np.ascontiguousarray(dir_delta[bd].T)                      # [R, B]

    in_maps = []
    for cid in range(NCORES):
        sl = slice(cid * LB, (cid + 1) * LB)
        in_maps.append({
            "ut": ut_h, "wm": wm_h, "vh": vh, "mwt": mwt_h,
            "mb": modulation_b,
            "stl": np.ascontiguousarray(stl_h[:, sl]),
            "ev": np.ascontiguousarray(ev_h[:, sl]),
            "sh": np.ascontiguousarray(batch_shifts[sl]),
            "x": np.ascontiguousarray(x[sl]),
        })

    nc = _get_program()
    trace = os.environ.get("BASS_KERNEL_TRACE", "") == "1"
    if trace:
        _install_ntff_hook()
    res = run_bass_kernel_spmd(nc, in_maps, list(range(NCORES)), trace=trace)
    if trace:
        kernel.last_exec_time_ns = res.exec_time_ns
    outs = [res.results[i]["out"] for i in range(NCORES)]
    return np.concatenate(outs, axis=0)


kernel.last_exec_time_ns = None
